# revision 31
# baseline (speedup 1.0000x reference)
"""Trainium2 (8 NeuronCores) kernel for nn_AdaptiveSliceSelector.

Strategy:
- Algebraic refold: GAT aggregation commutes with the per-branch weight
  matrix, so per-edge work happens in x-space (256-wide rows) and the
  weight matmuls (Wg@Ws folded into one 256x256 per branch) run on the
  aggregated output. Attention logits use folded vectors p=Wg@a_src,
  q=Wg@a_dst:  s = x@p, d = x@q,  w_e = exp(leaky_relu(s[src]+d[dst])).
  exp() without the max-shift is safe (logits ~ N(0,2)) and cancels in
  the softmax ratio.
- Self-loops are appended as ordinary edges on the host.
- Nodes are sharded round-robin-free: core c owns rows [c*R, (c+1)*R).
  The bf16 gather table (x rows + a validity flag column) is replicated
  to every core's HBM, so cross-partition edges need no halo exchange;
  per-edge rows are fetched with indirect DMA by global src index.
- Per (dst-tile, branch): gathered edge rows [128e, 272] become the
  matmul moving operand; the one-hot scatter matrix S[e, j] =
  w_e * (dstloc_e == j) is built on DVE/ACT and used as lhsT so the
  segment-softmax numerator/denominator come out of one PSUM matmul
  accumulation (the flag column yields the denominator, and host-side
  padding edges point at a zero row so they vanish from both).
- Strategy weights (softmax MLP over mean(x)) use a 1KB AllReduce.
"""

import os
import sys

sys.path.insert(0, "/opt/trn_rl_repo")

import numpy as np
import ml_dtypes

import concourse.bass as bass
import concourse.mybir as mybir
import concourse.tile as tile
from concourse import bacc
from concourse.bass_utils import run_bass_kernel_spmd

NCORES = 8
H = 256
P = 128
GROW = 272  # gather-table row width (bf16) -> 544B rows, 32B aligned
FLAG = H    # column holding the 1.0 validity flag
LN_EPS = 1e-5
NEG_SLOPE = 0.2

F32 = mybir.dt.float32
BF16 = mybir.dt.bfloat16
I32 = mybir.dt.int32
AO = mybir.AluOpType
AF = mybir.ActivationFunctionType

LAST_EXEC_NS = None  # stash for test harness
LAST_RES = None

_GRAPH_CACHE = {}


def _bf(a):
    return np.asarray(a, np.float32).astype(ml_dtypes.bfloat16)


# --------------------------------------------------------------------------
# device graph
# --------------------------------------------------------------------------

def _build(T, GR, cgs, NREAL, flags):
    """Build the SPMD Bass graph.

    T: dst tiles per core; GR: gather table rows; cgs: list of chunk
    counts in (t, b) order, len == 3*T; flags: zero/one-bias fast paths.
    """
    R = T * P
    totc = int(sum(cgs))
    cb_zero, gs_ones, bt_zero, gf_ones, bf_zero, btf_zero = flags

    nc = bacc.Bacc("TRN2", target_bir_lowering=False, debug=False,
                   num_devices=NCORES)

    gx = nc.dram_tensor("gx", [GR, GROW], BF16, kind="ExternalInput")
    xt_in = nc.dram_tensor("xt", [P, 2 * R], BF16, kind="ExternalInput")
    wc_in = nc.dram_tensor("wc", [P, 5 * 2 * H], BF16, kind="ExternalInput")
    prep_in = nc.dram_tensor("prep", [P, 3 * H], BF16, kind="ExternalInput")
    pq_in = nc.dram_tensor("pq", [P, 2 * 6], BF16, kind="ExternalInput")
    gsb_in = nc.dram_tensor("gsb", [P, 4 * H], F32, kind="ExternalInput")
    btb_in = nc.dram_tensor("btb", [P, 4 * H], F32, kind="ExternalInput")
    cbb_in = nc.dram_tensor("cbb", [P, 4 * H], F32, kind="ExternalInput")
    fin_in = nc.dram_tensor("fin", [P, 3 * H], F32, kind="ExternalInput")
    w1_in = nc.dram_tensor("w1", [P, 2 * P], F32, kind="ExternalInput")
    b1_in = nc.dram_tensor("b1", [P, 1], F32, kind="ExternalInput")
    w2_in = nc.dram_tensor("w2", [P, 4], F32, kind="ExternalInput")
    b2_in = nc.dram_tensor("b2", [P, 1], F32, kind="ExternalInput")
    aux_in = nc.dram_tensor("aux", [P, 2 * P], F32, kind="ExternalInput")
    auxb_in = nc.dram_tensor("auxb", [P, 2 * P], BF16, kind="ExternalInput")
    srcT_in = nc.dram_tensor("srcT", [P, totc], I32, kind="ExternalInput")
    dstT_in = nc.dram_tensor("dstT", [P, totc], BF16, kind="ExternalInput")
    xr_in = nc.dram_tensor("xr", [R, GROW], BF16, kind="ExternalInput")
    out = nc.dram_tensor("out", [R, H], F32, kind="ExternalOutput")

    with tile.TileContext(nc) as tc:
        with (
            tc.tile_pool(name="const", bufs=1) as cpool,
            tc.tile_pool(name="work", bufs=4) as work,
            tc.tile_pool(name="epi", bufs=2) as epi,
            tc.tile_pool(name="psum", bufs=2, space="PSUM") as psum,
            tc.tile_pool(name="psy", bufs=2, space="PSUM") as psy,
            tc.tile_pool(name="dram", bufs=1, space="DRAM") as dpool,
        ):
            # ---- resident loads ----
            xt = cpool.tile([P, 2, R], BF16)
            nc.sync.dma_start(out=xt[:], in_=xt_in.ap())
            wc = cpool.tile([P, 5, 2, H], BF16)
            nc.sync.dma_start(out=wc[:], in_=wc_in.ap())
            prepb = []
            for b in range(3):
                pb = cpool.tile([P, H], BF16, tag=f"prep{b}")
                nc.sync.dma_start(out=pb[:], in_=prep_in.ap()[:, b * H:(b + 1) * H])
                prepb.append(pb)
            pq = cpool.tile([P, 2, 6], BF16)
            nc.sync.dma_start(out=pq[:], in_=pq_in.ap())
            gsb = cpool.tile([P, 4, H], F32)
            nc.sync.dma_start(out=gsb[:], in_=gsb_in.ap())
            btb = cpool.tile([P, 4, H], F32)
            nc.sync.dma_start(out=btb[:], in_=btb_in.ap())
            cbb = cpool.tile([P, 4, H], F32)
            nc.sync.dma_start(out=cbb[:], in_=cbb_in.ap())
            fin = cpool.tile([P, 3, H], F32)
            nc.sync.dma_start(out=fin[:], in_=fin_in.ap())
            w1 = cpool.tile([P, 2, P], F32)
            nc.sync.dma_start(out=w1[:], in_=w1_in.ap())
            b1 = cpool.tile([P, 1], F32)
            nc.sync.dma_start(out=b1[:], in_=b1_in.ap())
            w2 = cpool.tile([P, 4], F32)
            nc.sync.dma_start(out=w2[:], in_=w2_in.ap())
            b2 = cpool.tile([P, 1], F32)
            nc.sync.dma_start(out=b2[:], in_=b2_in.ap())
            aux = cpool.tile([P, 2, P], F32)
            nc.sync.dma_start(out=aux[:], in_=aux_in.ap())
            idf = aux[:, 0, :]
            iota = aux[:, 1, :]
            idb = cpool.tile([P, P], BF16)
            nc.sync.dma_start(out=idb[:], in_=auxb_in.ap()[:, 0:P])
            iotab = cpool.tile([P, P], BF16)
            nc.sync.dma_start(out=iotab[:], in_=auxb_in.ap()[:, P:2 * P])
            epsc = cpool.tile([P, 1], F32)
            nc.gpsimd.memset(epsc[:], LN_EPS)
            srcT = cpool.tile([P, totc], I32)
            nc.sync.dma_start(out=srcT[:], in_=srcT_in.ap())
            dstT = cpool.tile([P, totc], BF16)
            nc.sync.dma_start(out=dstT[:], in_=dstT_in.ap())

            # ---- strategy weights: colsum -> AllReduce -> MLP -> sw ----
            cs = cpool.tile([P, 2], F32)
            nc.vector.tensor_reduce(out=cs[:], in_=xt[:],
                                    axis=mybir.AxisListType.X, op=AO.add)
            cin = dpool.tile([P, 2], F32)
            cout = dpool.tile([P, 2], F32)
            nc.gpsimd.dma_start(out=cin[:], in_=cs[:])
            nc.gpsimd.collective_compute(
                "AllReduce", AO.add,
                ins=[cin.opt()], outs=[cout.opt()],
                replica_groups=[list(range(NCORES))],
            )
            gsum = cpool.tile([P, 2], F32)
            nc.gpsimd.dma_start(out=gsum[:], in_=cout[:])
            gmean = cpool.tile([P, 2], F32)
            nc.vector.tensor_scalar_mul(gmean[:], gsum[:], 1.0 / NREAL)

            hps = psum.tile([P, 1], F32, tag="drp")
            for k in range(2):
                nc.tensor.matmul(hps[:], lhsT=w1[:, k, :], rhs=gmean[:, k:k + 1],
                                 start=(k == 0), stop=(k == 1))
            hsb = cpool.tile([P, 1], F32)
            nc.scalar.activation(hsb[:], hps[:], AF.Relu, bias=b1[:])
            lps = psum.tile([P, 4], F32, tag="drp")
            nc.tensor.matmul(lps[:4, :1], lhsT=w2[:], rhs=hsb[:],
                             start=True, stop=True)
            lsb = cpool.tile([P, 1], F32)
            nc.vector.tensor_tensor(out=lsb[:4, :], in0=lps[:4, :1],
                                    in1=b2[:4, :], op=AO.add)
            # transpose+replicate the 4 logits to all partitions
            lrp = psum.tile([P, 4], F32, tag="drp")
            nc.tensor.transpose(lrp[:, :4], lsb[:4, :1].to_broadcast([4, P]),
                                idf[:4, :4])
            esb = cpool.tile([P, 4], F32)
            nc.scalar.activation(esb[:], lrp[:, :4], AF.Exp)
            sesb = cpool.tile([P, 1], F32)
            nc.vector.tensor_reduce(out=sesb[:], in_=esb[:],
                                    axis=mybir.AxisListType.X, op=AO.add)
            rse = cpool.tile([P, 1], F32)
            nc.vector.reciprocal(rse[:], sesb[:])
            swrep = cpool.tile([P, 4], F32)
            nc.vector.tensor_scalar_mul(swrep[:], esb[:], rse[:])

            gssw = cpool.tile([P, 4, H], F32)
            btsw = cpool.tile([P, 4, H], F32)
            for b in range(4):
                nc.vector.tensor_scalar_mul(gssw[:, b, :], gsb[:, b, :],
                                            swrep[:, b:b + 1])
                nc.vector.tensor_scalar_mul(btsw[:, b, :], btb[:, b, :],
                                            swrep[:, b:b + 1])

            # ---- s,d = x_own @ [p|q]_b, row-major [128, 6, T] ----
            dsall = cpool.tile([P, 6, T], F32)
            for t in range(T):
                dps = psum.tile([P, 6], F32, tag="drp")
                for k in range(2):
                    nc.tensor.matmul(dps[:], lhsT=xt[:, k, t * P:(t + 1) * P],
                                     rhs=pq[:, k, :],
                                     start=(k == 0), stop=(k == 1))
                nc.vector.tensor_copy(out=dsall[:, :, t:t + 1],
                                      in_=dps[:, :, None])
            dsb16 = cpool.tile([P, 6, T], BF16)
            nc.vector.tensor_copy(out=dsb16[:], in_=dsall[:])

            # ---- epilogue A: z(psum) -> v sbuf + per-branch stats ----
            def epi_a(zps, b, vs, sums, sqs):
                if cb_zero[b]:
                    nc.scalar.activation(vs[:, b, :], zps[:], AF.Copy,
                                         accum_out=sums[:, b:b + 1])
                else:
                    nc.vector.tensor_tensor(out=vs[:, b, :], in0=zps[:],
                                            in1=cbb[:, b, :], op=AO.add)
                    nc.vector.tensor_reduce(out=sums[:, b:b + 1],
                                            in_=vs[:, b, :],
                                            axis=mybir.AxisListType.X,
                                            op=AO.add)
                sqd = epi.tile([P, H], F32, tag="sqd")
                nc.scalar.activation(sqd[:], vs[:, b, :], AF.Square,
                                     accum_out=sqs[:, b:b + 1])

            # ---- main loop over dst tile groups ----
            offs = np.concatenate([[0], np.cumsum(cgs)]).astype(int)
            GT = 4  # tiles per stats group

            swg = cpool.tile([P, 4 * GT], F32)
            for g in range(GT):
                nc.vector.tensor_copy(out=swg[:, 4 * g:4 * g + 4],
                                      in_=swrep[:])

            def branch_z(t, b, gi, gts, wself):
                CG = int(cgs[gi])
                off = int(offs[gi])
                gt = work.tile([P, CG, GROW], BF16, tag="gt")
                for cg in range(CG):
                    nc.gpsimd.indirect_dma_start(
                        out=gt[:, cg, :], out_offset=None,
                        in_=gx.ap(),
                        in_offset=bass.IndirectOffsetOnAxis(
                            ap=srcT[:, off + cg:off + cg + 1], axis=0),
                    )
                scr = work.tile([P, CG, H], BF16, tag="scr")
                nc.vector.tensor_tensor(
                    out=scr[:], in0=gt[:, :, 0:H],
                    in1=prepb[b][:, None, :].to_broadcast([P, CG, H]),
                    op=AO.mult)
                ssc = work.tile([P, CG], F32, tag="ssc")
                nc.vector.tensor_reduce(out=ssc[:], in_=scr[:],
                                        axis=mybir.AxisListType.X, op=AO.add)
                ssb = work.tile([P, CG], BF16, tag="ssb")
                nc.vector.tensor_copy(out=ssb[:], in_=ssc[:])
                drp = psum.tile([P, P], BF16, tag="drp")
                nc.tensor.transpose(
                    drp[:], dsb16[:, 3 + b, t:t + 1].to_broadcast([P, P]), idb)
                dre = work.tile([P, P], BF16, tag="dre")
                nc.scalar.copy(dre[:], drp[:])
                tw = work.tile([P, CG, P], BF16, tag="tw")
                nc.vector.tensor_tensor(
                    out=tw[:],
                    in0=dre[:, None, :].to_broadcast([P, CG, P]),
                    in1=ssb[:, :, None].to_broadcast([P, CG, P]),
                    op=AO.add)
                t02 = work.tile([P, CG, P], BF16, tag="t02")
                nc.vector.tensor_scalar_mul(t02[:], tw[:], NEG_SLOPE)
                tm = work.tile([P, CG, P], BF16, tag="tm")
                nc.vector.tensor_tensor(out=tm[:], in0=tw[:], in1=t02[:],
                                        op=AO.max)
                ex = work.tile([P, CG, P], BF16, tag="ex")
                nc.scalar.activation(ex[:], tm[:], AF.Exp)
                oh = work.tile([P, CG, P], BF16, tag="oh")
                nc.vector.tensor_tensor(
                    out=oh[:],
                    in0=dstT[:, off:off + CG, None].to_broadcast([P, CG, P]),
                    in1=iotab[:, None, :].to_broadcast([P, CG, P]),
                    op=AO.is_equal)
                sm_ = work.tile([P, CG, P], BF16, tag="smat")
                nc.vector.tensor_tensor(out=sm_[:], in0=ex[:], in1=oh[:],
                                        op=AO.mult)
                ssf = work.tile([P, P], BF16, tag="ssf")
                nc.vector.tensor_scalar_mul(ssf[:], idb, wself[:, b:b + 1])
                yps = psy.tile([P, GROW], F32, tag="yps")
                for cg in range(CG):
                    nc.tensor.matmul(yps[:], lhsT=sm_[:, cg, :],
                                     rhs=gt[:, cg, :],
                                     start=(cg == 0), stop=False)
                nc.tensor.matmul(yps[:], lhsT=ssf[:], rhs=gts[:],
                                 start=False, stop=True)
                rec = work.tile([P, 1], F32, tag="rec")
                nc.vector.reciprocal(rec[:], yps[:, FLAG:FLAG + 1])
                y = work.tile([P, H], BF16, tag="y")
                nc.vector.tensor_scalar_mul(y[:], yps[:, 0:H], rec[:])
                yT = work.tile([P, 2, P], BF16, tag="yT")
                for k in range(2):
                    tps = psum.tile([P, P], BF16, tag="tps")
                    nc.tensor.transpose(tps[:], y[:, k * P:(k + 1) * P], idb)
                    nc.vector.tensor_copy(out=yT[:, k, :], in_=tps[:])
                zps = psy.tile([P, H], F32, tag="zps")
                for k in range(2):
                    nc.tensor.matmul(zps[:], lhsT=yT[:, k, :],
                                     rhs=wc[:, b, k, :],
                                     start=(k == 0), stop=(k == 1))
                return zps

            def heron(vrs, w, tagp):
                # sqrt via 3 Heron iterations, then reciprocal
                sh = epi.tile([P, w], F32, tag=tagp + "sh")
                nc.vector.tensor_scalar(sh[:], vrs[:], 0.5, 0.5,
                                        AO.mult, AO.add)
                for _ in range(3):
                    hu = epi.tile([P, w], F32, tag=tagp + "hu")
                    nc.vector.reciprocal(hu[:], sh[:])
                    ht = epi.tile([P, w], F32, tag=tagp + "ht")
                    nc.vector.tensor_tensor(out=ht[:], in0=vrs[:], in1=hu[:],
                                            op=AO.mult)
                    h2 = epi.tile([P, w], F32, tag=tagp + "h2")
                    nc.vector.tensor_tensor(out=h2[:], in0=sh[:], in1=ht[:],
                                            op=AO.add)
                    sh = epi.tile([P, w], F32, tag=tagp + "sh")
                    nc.vector.tensor_scalar_mul(sh[:], h2[:], 0.5)
                ri = epi.tile([P, w], F32, tag=tagp + "ri")
                nc.vector.reciprocal(ri[:], sh[:])
                return ri

            gi = 0
            t0 = 0
            while t0 < T:
                NG = min(GT, T - t0)
                W4 = 4 * NG
                sums = epi.tile([P, 4 * GT], F32, tag="sums")
                sqs = epi.tile([P, 4 * GT], F32, tag="sqs")
                vss = []
                wselfs = []
                for g in range(NG):
                    t = t0 + g
                    gts = work.tile([P, GROW], BF16, tag="gts")
                    nc.sync.dma_start(out=gts[:],
                                      in_=xr_in.ap()[t * P:(t + 1) * P, :])
                    ws1 = work.tile([P, 3], F32, tag="ws1")
                    nc.vector.tensor_tensor(out=ws1[:],
                                            in0=dsall[:, 0:3, t:t + 1],
                                            in1=dsall[:, 3:6, t:t + 1],
                                            op=AO.add)
                    ws2 = work.tile([P, 3], F32, tag="ws2")
                    nc.vector.tensor_scalar_mul(ws2[:], ws1[:], NEG_SLOPE)
                    ws3 = work.tile([P, 3], F32, tag="ws3")
                    nc.vector.tensor_tensor(out=ws3[:], in0=ws1[:],
                                            in1=ws2[:], op=AO.max)
                    wself = work.tile([P, 3], F32, tag="wself")
                    nc.scalar.activation(wself[:], ws3[:], AF.Exp)
                    wselfs.append(wself)

                    vs = epi.tile([P, 4, H], F32, tag=f"vs{g}")
                    vss.append(vs)
                    for b in range(3):
                        zps = branch_z(t, b, gi, gts, wself)
                        gi += 1
                        epi_a(zps, b, vs, sums[:, 4 * g:],
                              sqs[:, 4 * g:])
                    zps = psy.tile([P, H], F32, tag="zps")
                    for k in range(2):
                        nc.tensor.matmul(
                            zps[:], lhsT=xt[:, k, t * P:(t + 1) * P],
                            rhs=wc[:, 3, k, :], start=(k == 0), stop=(k == 1))
                    epi_a(zps, 3, vs, sums[:, 4 * g:], sqs[:, 4 * g:])

                # batched LN stats for NG tiles x 4 branches
                mus = epi.tile([P, 4 * GT], F32, tag="mus")
                nc.vector.tensor_scalar_mul(mus[:, :W4], sums[:, :W4],
                                            1.0 / H)
                m2s = epi.tile([P, 4 * GT], F32, tag="m2s")
                nc.vector.tensor_scalar(m2s[:, :W4], sqs[:, :W4], 1.0 / H,
                                        LN_EPS, AO.mult, AO.add)
                mqs = epi.tile([P, 4 * GT], F32, tag="mqs")
                nc.scalar.activation(mqs[:, :W4], mus[:, :W4], AF.Square)
                vrs = epi.tile([P, 4 * GT], F32, tag="vrs")
                nc.vector.tensor_tensor(out=vrs[:, :W4], in0=m2s[:, :W4],
                                        in1=mqs[:, :W4], op=AO.subtract)
                ris = heron(vrs[:, :W4], W4, "b")
                rsw = epi.tile([P, 4 * GT], F32, tag="rsw")
                nc.vector.tensor_tensor(out=rsw[:, :W4], in0=ris[:],
                                        in1=swg[:, :W4], op=AO.mult)
                nbm = epi.tile([P, 4 * GT], F32, tag="nbm")
                nc.vector.tensor_tensor(out=nbm[:, :W4], in0=mus[:, :W4],
                                        in1=rsw[:, :W4], op=AO.mult)
                nc.vector.tensor_scalar_mul(nbm[:, :W4], nbm[:, :W4], -1.0)

                # normalize + relu + combine + fusion matmul per tile
                fvs = []
                fsqs = epi.tile([P, GT], F32, tag="fsqs")
                fsumcat = epi.tile([P, GT], F32, tag="fsumcat")
                for g in range(NG):
                    t = t0 + g
                    vs = vss[g]
                    comb = work.tile([P, H], F32, tag="comb")
                    for b in range(4):
                        c0 = 4 * g + b
                        t1 = epi.tile([P, H], F32, tag="t1")
                        if gs_ones[b]:
                            nc.scalar.activation(t1[:], vs[:, b, :],
                                                 AF.Identity,
                                                 bias=nbm[:, c0:c0 + 1],
                                                 scale=rsw[:, c0:c0 + 1])
                        else:
                            nc.vector.tensor_scalar(t1[:], vs[:, b, :],
                                                    mus[:, c0:c0 + 1],
                                                    ris[:, c0:c0 + 1],
                                                    AO.subtract, AO.mult)
                        cur = t1
                        if not gs_ones[b]:
                            t2 = epi.tile([P, H], F32, tag="t2")
                            nc.vector.tensor_tensor(out=t2[:], in0=cur[:],
                                                    in1=gssw[:, b, :],
                                                    op=AO.mult)
                            cur = t2
                        if not bt_zero[b]:
                            t3 = epi.tile([P, H], F32, tag="t3")
                            nc.vector.tensor_tensor(out=t3[:], in0=cur[:],
                                                    in1=btsw[:, b, :],
                                                    op=AO.add)
                            cur = t3
                        if b == 0:
                            nc.scalar.activation(comb[:], cur[:], AF.Relu)
                        else:
                            t4 = epi.tile([P, H], F32, tag="t4")
                            nc.scalar.activation(t4[:], cur[:], AF.Relu)
                            nc.vector.tensor_tensor(out=comb[:], in0=comb[:],
                                                    in1=t4[:], op=AO.add)
                    cb16 = work.tile([P, H], BF16, tag="cb16")
                    nc.vector.tensor_copy(out=cb16[:], in_=comb[:])
                    cT = work.tile([P, 2, P], BF16, tag="cT")
                    for k in range(2):
                        tps = psum.tile([P, P], BF16, tag="tps")
                        nc.tensor.transpose(tps[:], cb16[:, k * P:(k + 1) * P],
                                            idb)
                        nc.vector.tensor_copy(out=cT[:, k, :], in_=tps[:])
                    fps = psy.tile([P, H], F32, tag="zps")
                    for k in range(2):
                        nc.tensor.matmul(fps[:], lhsT=cT[:, k, :],
                                         rhs=wc[:, 4, k, :],
                                         start=(k == 0), stop=(k == 1))
                    fv = epi.tile([P, H], F32, tag=f"fv{g}")
                    fvs.append(fv)
                    if bf_zero:
                        nc.scalar.activation(fv[:], fps[:], AF.Copy,
                                             accum_out=fsumcat[:, g:g + 1])
                    else:
                        nc.vector.tensor_tensor(out=fv[:], in0=fps[:],
                                                in1=fin[:, 2, :], op=AO.add)
                        nc.vector.tensor_reduce(out=fsumcat[:, g:g + 1],
                                                in_=fv[:],
                                                axis=mybir.AxisListType.X,
                                                op=AO.add)
                    sqd2 = epi.tile([P, H], F32, tag="sqd")
                    nc.scalar.activation(sqd2[:], fv[:], AF.Square,
                                         accum_out=fsqs[:, g:g + 1])

                # batched final-LN stats
                fmus = epi.tile([P, GT], F32, tag="fmus")
                nc.vector.tensor_scalar_mul(fmus[:, :NG], fsumcat[:, :NG],
                                            1.0 / H)
                fm2 = epi.tile([P, GT], F32, tag="fm2")
                nc.vector.tensor_scalar(fm2[:, :NG], fsqs[:, :NG], 1.0 / H,
                                        LN_EPS, AO.mult, AO.add)
                fmq = epi.tile([P, GT], F32, tag="fmq")
                nc.scalar.activation(fmq[:, :NG], fmus[:, :NG], AF.Square)
                fvr = epi.tile([P, GT], F32, tag="fvr")
                nc.vector.tensor_tensor(out=fvr[:, :NG], in0=fm2[:, :NG],
                                        in1=fmq[:, :NG], op=AO.subtract)
                fri = heron(fvr[:, :NG], NG, "f")
                nfm = epi.tile([P, GT], F32, tag="nfm")
                nc.vector.tensor_tensor(out=nfm[:, :NG], in0=fmus[:, :NG],
                                        in1=fri[:, :NG], op=AO.mult)
                nc.vector.tensor_scalar_mul(nfm[:, :NG], nfm[:, :NG], -1.0)
                for g in range(NG):
                    t = t0 + g
                    ft1 = epi.tile([P, H], F32, tag="ft1")
                    nc.scalar.activation(ft1[:], fvs[g][:], AF.Identity,
                                         bias=nfm[:, g:g + 1],
                                         scale=fri[:, g:g + 1])
                    fcur = ft1
                    if not gf_ones:
                        ft2 = epi.tile([P, H], F32, tag="ft2")
                        nc.vector.tensor_tensor(out=ft2[:], in0=fcur[:],
                                                in1=fin[:, 0, :], op=AO.mult)
                        fcur = ft2
                    if not btf_zero:
                        ft3 = epi.tile([P, H], F32, tag="ft3")
                        nc.vector.tensor_tensor(out=ft3[:], in0=fcur[:],
                                                in1=fin[:, 1, :], op=AO.add)
                        fcur = ft3
                    osb = work.tile([P, H], F32, tag="osb")
                    nc.scalar.activation(osb[:], fcur[:], AF.Relu)
                    nc.sync.dma_start(out=out.ap()[t * P:(t + 1) * P, :],
                                      in_=osb[:])
                t0 += NG

    nc.compile()
    return nc


# --------------------------------------------------------------------------
# host side
# --------------------------------------------------------------------------

def kernel(x, edge_index, edge_attr, Wg, a_src, a_dst, bg, Ws, bs, gs, betas,
           W1, b1, W2, b2, Wf, bf, gf, betaf):
    global LAST_EXEC_NS
    x = np.asarray(x, np.float32)
    N = x.shape[0]
    R = int(np.ceil(N / NCORES / P)) * P
    T = R // P
    NPAD = NCORES * R
    DUMMY_PAD = NPAD
    DUMMY_SELF = NPAD + 1
    GR = int(np.ceil((NPAD + 2) / P)) * P

    Wg = np.asarray(Wg, np.float64)
    a_src_ = np.asarray(a_src, np.float64)
    a_dst_ = np.asarray(a_dst, np.float64)
    bg = np.asarray(bg, np.float64)
    Ws_ = np.asarray(Ws, np.float64)
    bs_ = np.asarray(bs, np.float64)

    p = np.stack([Wg[i] @ a_src_[i] for i in range(3)])
    q = np.stack([Wg[i] @ a_dst_[i] for i in range(3)])
    C = np.stack([Wg[i] @ Ws_[i] for i in range(3)])
    cb = np.stack([bg[i] @ Ws_[i] + bs_[i] for i in range(3)])

    # gather table (replicated)
    gxt = np.zeros((GR, GROW), dtype=ml_dtypes.bfloat16)
    gxt[:N, :H] = _bf(x)
    gxt[:N, FLAG] = 1.0
    gxt[DUMMY_SELF, FLAG] = 1.0

    # ---- edge bucketing ----
    src = np.asarray(edge_index)[0].astype(np.int64)
    dst = np.asarray(edge_index)[1].astype(np.int64)
    attr = np.asarray(edge_attr).astype(np.int64)
    keep = attr < 3
    ks = src[keep]
    kd = dst[keep]
    ka = attr[keep]
    core_of = kd // R
    tl = (kd - core_of * R) // P
    jl = (kd - core_of * R) % P
    # group id = ((core*T + t)*3 + b)
    gid = (core_of * T + tl) * 3 + ka
    order = np.argsort(gid, kind="stable")
    gid_s, ks_s, jl_s = gid[order], ks[order], jl[order]
    counts = np.bincount(gid_s, minlength=NCORES * T * 3).reshape(NCORES, T, 3)
    bounds = np.concatenate([[0], np.cumsum(
        counts.reshape(-1))]).astype(np.int64)

    # chunk counts per (t, b): max over cores (self-loops go direct)
    cgs = np.maximum(np.ceil(counts.max(axis=0) / P), 1).astype(np.int64)
    cgs_tb = cgs.reshape(-1)  # (t, b) order
    totc = int(cgs_tb.sum())
    offs = np.concatenate([[0], np.cumsum(cgs_tb)]).astype(np.int64)

    srcT = np.full((NCORES, P, totc), DUMMY_PAD, dtype=np.int32)
    dstT = np.zeros((NCORES, P, totc), dtype=np.float32)
    for c in range(NCORES):
        for t in range(T):
            for b in range(3):
                g = (c * T + t) * 3 + b
                lo, hi = bounds[g], bounds[g + 1]
                e_src = ks_s[lo:hi]
                e_jl = jl_s[lo:hi]
                off = offs[t * 3 + b]
                CG = cgs[t, b]
                ne = len(e_src)
                buf_s = np.full(CG * P, DUMMY_PAD, dtype=np.int64)
                buf_j = np.zeros(CG * P, dtype=np.int64)
                buf_s[:ne] = e_src
                buf_j[:ne] = e_jl
                srcT[c, :, off:off + CG] = buf_s.reshape(CG, P).T
                dstT[c, :, off:off + CG] = buf_j.reshape(CG, P).T

    # ---- constant packs ----
    def rep(v):  # replicate a [H] vector across partitions
        return np.tile(np.asarray(v, np.float32)[None, :], (P, 1))

    wcs = [C[0], C[1], C[2], np.asarray(Ws_[3]), np.asarray(Wf, np.float64)]
    wc = np.zeros((P, 5, 2, H), dtype=ml_dtypes.bfloat16)
    for ci, M in enumerate(wcs):
        for k in range(2):
            wc[:, ci, k, :] = _bf(M[k * P:(k + 1) * P, :])
    prep = np.zeros((P, 3, H), dtype=ml_dtypes.bfloat16)
    for b in range(3):
        prep[:, b, :] = _bf(p[b])[None, :]
    pqa = np.zeros((P, 2, 6), dtype=ml_dtypes.bfloat16)
    for k in range(2):
        for j in range(3):
            pqa[:, k, j] = _bf(p[j][k * P:(k + 1) * P])
            pqa[:, k, 3 + j] = _bf(q[j][k * P:(k + 1) * P])
    gs4 = np.stack([rep(np.asarray(gs)[b]) for b in range(4)], axis=1)
    bt4 = np.stack([rep(np.asarray(betas)[b]) for b in range(4)], axis=1)
    cb4 = np.stack([rep(cb[0]), rep(cb[1]), rep(cb[2]),
                    rep(np.asarray(bs)[3])], axis=1)
    fin = np.stack([rep(gf), rep(betaf), rep(bf)], axis=1)
    w1a = np.zeros((P, 2, P), np.float32)
    W1a = np.asarray(W1, np.float32)
    for k in range(2):
        w1a[:, k, :] = W1a[k * P:(k + 1) * P, :]
    b1a = np.asarray(b1, np.float32).reshape(P, 1)
    w2a = np.asarray(W2, np.float32)  # [128, 4]
    b2a = np.zeros((P, 1), np.float32)
    b2a[:4, 0] = np.asarray(b2, np.float32)
    auxa = np.zeros((P, 2, P), np.float32)
    auxa[:, 0, :] = np.eye(P, dtype=np.float32)
    auxa[:, 1, :] = np.arange(P, dtype=np.float32)[None, :]
    auxb = np.zeros((P, 2, P), dtype=ml_dtypes.bfloat16)
    auxb[:, 0, :] = np.eye(P, dtype=np.float32)
    auxb[:, 1, :] = np.arange(P, dtype=np.float32)[None, :]

    xpad = np.zeros((NPAD, H), np.float32)
    xpad[:N] = x

    cb_zero = tuple(bool(np.all(cb4[:, b, :] == 0)) for b in range(4))
    gs_ones = tuple(bool(np.all(gs4[:, b, :] == 1)) for b in range(4))
    bt_zero = tuple(bool(np.all(bt4[:, b, :] == 0)) for b in range(4))
    gf_ones = bool(np.all(fin[:, 0, :] == 1))
    btf_zero = bool(np.all(fin[:, 1, :] == 0))
    bf_zero = bool(np.all(fin[:, 2, :] == 0))
    flags = (cb_zero, gs_ones, bt_zero, gf_ones, bf_zero, btf_zero)

    key = (T, GR, N, tuple(int(v) for v in cgs_tb), flags)
    if key not in _GRAPH_CACHE:
        _GRAPH_CACHE[key] = _build(T, GR, list(cgs_tb), N, flags)
    nc = _GRAPH_CACHE[key]

    in_maps = []
    for c in range(NCORES):
        xo = xpad[c * R:(c + 1) * R]  # [R, H]
        xtc = np.zeros((P, 2, R), dtype=ml_dtypes.bfloat16)
        xoT = _bf(xo).T  # [H, R]
        for k in range(2):
            xtc[:, k, :] = xoT[k * P:(k + 1) * P, :]
        xrc = gxt[c * R:(c + 1) * R].copy()
        xrc[:, FLAG] = 1.0
        in_maps.append({
            "gx": gxt,
            "xr": xrc,
            "xt": xtc.reshape(P, 2 * R),
            "wc": wc.reshape(P, 5 * 2 * H),
            "prep": prep.reshape(P, 3 * H),
            "pq": pqa.reshape(P, 2 * 6),
            "gsb": gs4.reshape(P, 4 * H),
            "btb": bt4.reshape(P, 4 * H),
            "cbb": cb4.reshape(P, 4 * H),
            "fin": fin.reshape(P, 3 * H),
            "w1": w1a.reshape(P, 2 * P),
            "b1": b1a,
            "w2": w2a,
            "b2": b2a,
            "aux": auxa.reshape(P, 2 * P),
            "auxb": auxb.reshape(P, 2 * P),
            "srcT": srcT[c],
            "dstT": dstT[c].astype(ml_dtypes.bfloat16),
        })

    trace = os.environ.get("KERNEL_TRACE", "0") == "1"
    res = run_bass_kernel_spmd(nc, in_maps, core_ids=list(range(NCORES)),
                               trace=trace)
    LAST_EXEC_NS = res.exec_time_ns
    global LAST_RES
    LAST_RES = res
    full = np.concatenate([res.results[c]["out"] for c in range(NCORES)],
                          axis=0)
    return full[:N].astype(np.float32)


# revision 32
# speedup vs baseline: 1.0689x; 1.0689x over previous
"""Trainium2 (8 NeuronCores) kernel for nn_AdaptiveSliceSelector.

Strategy:
- Algebraic refold: GAT aggregation commutes with the per-branch weight
  matrix, so per-edge work happens in x-space (256-wide rows) and the
  weight matmuls (Wg@Ws folded into one 256x256 per branch) run on the
  aggregated output. Attention logits use folded vectors p=Wg@a_src,
  q=Wg@a_dst:  s = x@p, d = x@q,  w_e = exp(leaky_relu(s[src]+d[dst])).
  exp() without the max-shift is safe (logits ~ N(0,2)) and cancels in
  the softmax ratio.
- Self-loops are appended as ordinary edges on the host.
- Nodes are sharded round-robin-free: core c owns rows [c*R, (c+1)*R).
  The bf16 gather table (x rows + a validity flag column) is replicated
  to every core's HBM, so cross-partition edges need no halo exchange;
  per-edge rows are fetched with indirect DMA by global src index.
- Per (dst-tile, branch): gathered edge rows [128e, 272] become the
  matmul moving operand; the one-hot scatter matrix S[e, j] =
  w_e * (dstloc_e == j) is built on DVE/ACT and used as lhsT so the
  segment-softmax numerator/denominator come out of one PSUM matmul
  accumulation (the flag column yields the denominator, and host-side
  padding edges point at a zero row so they vanish from both).
- Strategy weights (softmax MLP over mean(x)) use a 1KB AllReduce.
"""

import os
import sys

sys.path.insert(0, "/opt/trn_rl_repo")

import numpy as np
import ml_dtypes

import concourse.bass as bass
import concourse.mybir as mybir
import concourse.tile as tile
from concourse import bacc
from concourse.bass_utils import run_bass_kernel_spmd

NCORES = 8
H = 256
P = 128
GROW = 272  # gather-table row width (bf16) -> 544B rows, 32B aligned
FLAG = H    # column holding the 1.0 validity flag
LN_EPS = 1e-5
NEG_SLOPE = 0.2

F32 = mybir.dt.float32
BF16 = mybir.dt.bfloat16
I32 = mybir.dt.int32
AO = mybir.AluOpType
AF = mybir.ActivationFunctionType

LAST_EXEC_NS = None  # stash for test harness
LAST_RES = None

_GRAPH_CACHE = {}


def _bf(a):
    return np.asarray(a, np.float32).astype(ml_dtypes.bfloat16)


# --------------------------------------------------------------------------
# device graph
# --------------------------------------------------------------------------

def _build(T, GR, cgs, NREAL, flags):
    """Build the SPMD Bass graph.

    T: dst tiles per core; GR: gather table rows; cgs: list of chunk
    counts in (t, b) order, len == 3*T; flags: zero/one-bias fast paths.
    """
    R = T * P
    totc = int(sum(cgs))
    cb_zero, gs_ones, bt_zero, gf_ones, bf_zero, btf_zero = flags

    nc = bacc.Bacc("TRN2", target_bir_lowering=False, debug=False,
                   num_devices=NCORES)

    gx = nc.dram_tensor("gx", [GR, GROW], BF16, kind="ExternalInput")
    xt_in = nc.dram_tensor("xt", [P, 2 * R], BF16, kind="ExternalInput")
    wc_in = nc.dram_tensor("wc", [P, 5 * 2 * H], BF16, kind="ExternalInput")
    prep_in = nc.dram_tensor("prep", [P, 3 * H], BF16, kind="ExternalInput")
    pq_in = nc.dram_tensor("pq", [P, 2 * 6], BF16, kind="ExternalInput")
    gsb_in = nc.dram_tensor("gsb", [P, 4 * H], F32, kind="ExternalInput")
    btb_in = nc.dram_tensor("btb", [P, 4 * H], F32, kind="ExternalInput")
    cbb_in = nc.dram_tensor("cbb", [P, 4 * H], F32, kind="ExternalInput")
    fin_in = nc.dram_tensor("fin", [P, 3 * H], F32, kind="ExternalInput")
    w1_in = nc.dram_tensor("w1", [P, 2 * P], F32, kind="ExternalInput")
    b1_in = nc.dram_tensor("b1", [P, 1], F32, kind="ExternalInput")
    w2_in = nc.dram_tensor("w2", [P, 4], F32, kind="ExternalInput")
    b2_in = nc.dram_tensor("b2", [P, 1], F32, kind="ExternalInput")
    aux_in = nc.dram_tensor("aux", [P, 2 * P], F32, kind="ExternalInput")
    auxb_in = nc.dram_tensor("auxb", [P, 2 * P], BF16, kind="ExternalInput")
    srcT_in = nc.dram_tensor("srcT", [P, totc], I32, kind="ExternalInput")
    dstT_in = nc.dram_tensor("dstT", [P, totc], BF16, kind="ExternalInput")
    xr_in = nc.dram_tensor("xr", [R, GROW], BF16, kind="ExternalInput")
    out = nc.dram_tensor("out", [R, H], F32, kind="ExternalOutput")

    with tile.TileContext(nc) as tc:
        with (
            tc.tile_pool(name="const", bufs=1) as cpool,
            tc.tile_pool(name="work", bufs=4) as work,
            tc.tile_pool(name="epi", bufs=2) as epi,
            tc.tile_pool(name="psum", bufs=2, space="PSUM") as psum,
            tc.tile_pool(name="psy", bufs=2, space="PSUM") as psy,
            tc.tile_pool(name="dram", bufs=1, space="DRAM") as dpool,
        ):
            # ---- resident loads ----
            xt = cpool.tile([P, 2, R], BF16)
            nc.sync.dma_start(out=xt[:], in_=xt_in.ap())
            wc = cpool.tile([P, 5, 2, H], BF16)
            nc.sync.dma_start(out=wc[:], in_=wc_in.ap())
            prepb = []
            for b in range(3):
                pb = cpool.tile([P, H], BF16, tag=f"prep{b}")
                nc.sync.dma_start(out=pb[:], in_=prep_in.ap()[:, b * H:(b + 1) * H])
                prepb.append(pb)
            pq = cpool.tile([P, 2, 6], BF16)
            nc.sync.dma_start(out=pq[:], in_=pq_in.ap())
            gsb = cpool.tile([P, 4, H], F32)
            nc.sync.dma_start(out=gsb[:], in_=gsb_in.ap())
            btb = cpool.tile([P, 4, H], F32)
            nc.sync.dma_start(out=btb[:], in_=btb_in.ap())
            cbb = cpool.tile([P, 4, H], F32)
            nc.sync.dma_start(out=cbb[:], in_=cbb_in.ap())
            fin = cpool.tile([P, 3, H], F32)
            nc.sync.dma_start(out=fin[:], in_=fin_in.ap())
            w1 = cpool.tile([P, 2, P], F32)
            nc.sync.dma_start(out=w1[:], in_=w1_in.ap())
            b1 = cpool.tile([P, 1], F32)
            nc.sync.dma_start(out=b1[:], in_=b1_in.ap())
            w2 = cpool.tile([P, 4], F32)
            nc.sync.dma_start(out=w2[:], in_=w2_in.ap())
            b2 = cpool.tile([P, 1], F32)
            nc.sync.dma_start(out=b2[:], in_=b2_in.ap())
            aux = cpool.tile([P, 2, P], F32)
            nc.sync.dma_start(out=aux[:], in_=aux_in.ap())
            idf = aux[:, 0, :]
            iota = aux[:, 1, :]
            idb = cpool.tile([P, P], BF16)
            nc.sync.dma_start(out=idb[:], in_=auxb_in.ap()[:, 0:P])
            iotab = cpool.tile([P, P], BF16)
            nc.sync.dma_start(out=iotab[:], in_=auxb_in.ap()[:, P:2 * P])
            epsc = cpool.tile([P, 1], F32)
            nc.gpsimd.memset(epsc[:], LN_EPS)
            srcT = cpool.tile([P, totc], I32)
            nc.sync.dma_start(out=srcT[:], in_=srcT_in.ap())
            dstT = cpool.tile([P, totc], BF16)
            nc.sync.dma_start(out=dstT[:], in_=dstT_in.ap())

            # ---- strategy weights: colsum -> AllReduce -> MLP -> sw ----
            cs = cpool.tile([P, 2], F32)
            nc.vector.tensor_reduce(out=cs[:], in_=xt[:],
                                    axis=mybir.AxisListType.X, op=AO.add)
            cin = dpool.tile([P, 2], F32)
            cout = dpool.tile([P, 2], F32)
            nc.gpsimd.dma_start(out=cin[:], in_=cs[:])
            nc.gpsimd.collective_compute(
                "AllReduce", AO.add,
                ins=[cin.opt()], outs=[cout.opt()],
                replica_groups=[list(range(NCORES))],
            )
            gsum = cpool.tile([P, 2], F32)
            nc.gpsimd.dma_start(out=gsum[:], in_=cout[:])
            gmean = cpool.tile([P, 2], F32)
            nc.vector.tensor_scalar_mul(gmean[:], gsum[:], 1.0 / NREAL)

            hps = psum.tile([P, 1], F32, tag="drp")
            for k in range(2):
                nc.tensor.matmul(hps[:], lhsT=w1[:, k, :], rhs=gmean[:, k:k + 1],
                                 start=(k == 0), stop=(k == 1))
            hsb = cpool.tile([P, 1], F32)
            nc.scalar.activation(hsb[:], hps[:], AF.Relu, bias=b1[:])
            lps = psum.tile([P, 4], F32, tag="drp")
            nc.tensor.matmul(lps[:4, :1], lhsT=w2[:], rhs=hsb[:],
                             start=True, stop=True)
            lsb = cpool.tile([P, 1], F32)
            nc.vector.tensor_tensor(out=lsb[:4, :], in0=lps[:4, :1],
                                    in1=b2[:4, :], op=AO.add)
            # transpose+replicate the 4 logits to all partitions
            lrp = psum.tile([P, 4], F32, tag="drp")
            nc.tensor.transpose(lrp[:, :4], lsb[:4, :1].to_broadcast([4, P]),
                                idf[:4, :4])
            esb = cpool.tile([P, 4], F32)
            nc.scalar.activation(esb[:], lrp[:, :4], AF.Exp)
            sesb = cpool.tile([P, 1], F32)
            nc.vector.tensor_reduce(out=sesb[:], in_=esb[:],
                                    axis=mybir.AxisListType.X, op=AO.add)
            rse = cpool.tile([P, 1], F32)
            nc.vector.reciprocal(rse[:], sesb[:])
            swrep = cpool.tile([P, 4], F32)
            nc.vector.tensor_scalar_mul(swrep[:], esb[:], rse[:])

            gssw = cpool.tile([P, 4, H], F32)
            btsw = cpool.tile([P, 4, H], F32)
            for b in range(4):
                nc.vector.tensor_scalar_mul(gssw[:, b, :], gsb[:, b, :],
                                            swrep[:, b:b + 1])
                nc.vector.tensor_scalar_mul(btsw[:, b, :], btb[:, b, :],
                                            swrep[:, b:b + 1])

            # ---- s,d = x_own @ [p|q]_b, row-major [128, 6, T] ----
            dsall = cpool.tile([P, 6, T], F32)
            for t in range(T):
                dps = psum.tile([P, 6], F32, tag="drp")
                for k in range(2):
                    nc.tensor.matmul(dps[:], lhsT=xt[:, k, t * P:(t + 1) * P],
                                     rhs=pq[:, k, :],
                                     start=(k == 0), stop=(k == 1))
                nc.vector.tensor_copy(out=dsall[:, :, t:t + 1],
                                      in_=dps[:, :, None])
            dsb16 = cpool.tile([P, 6, T], BF16)
            nc.vector.tensor_copy(out=dsb16[:], in_=dsall[:])

            # ---- epilogue A: z(psum) -> v sbuf + per-branch stats ----
            def epi_a(zps, b, vs, sums, sqs):
                if cb_zero[b]:
                    nc.scalar.activation(vs[:, b, :], zps[:], AF.Copy,
                                         accum_out=sums[:, b:b + 1])
                else:
                    nc.vector.tensor_tensor(out=vs[:, b, :], in0=zps[:],
                                            in1=cbb[:, b, :], op=AO.add)
                    nc.vector.tensor_reduce(out=sums[:, b:b + 1],
                                            in_=vs[:, b, :],
                                            axis=mybir.AxisListType.X,
                                            op=AO.add)
                sqd = epi.tile([P, H], F32, tag="sqd")
                nc.scalar.activation(sqd[:], vs[:, b, :], AF.Square,
                                     accum_out=sqs[:, b:b + 1])

            # ---- main loop over dst tile groups ----
            offs = np.concatenate([[0], np.cumsum(cgs)]).astype(int)
            GT = 4  # tiles per stats group

            swg = cpool.tile([P, 4 * GT], F32)
            for g in range(GT):
                nc.vector.tensor_copy(out=swg[:, 4 * g:4 * g + 4],
                                      in_=swrep[:])

            def branch_z(t, b, gi, gts, wself):
                CG = int(cgs[gi])
                off = int(offs[gi])
                gt = work.tile([P, CG, GROW], BF16, tag="gt")
                for cg in range(CG):
                    nc.gpsimd.indirect_dma_start(
                        out=gt[:, cg, :], out_offset=None,
                        in_=gx.ap(),
                        in_offset=bass.IndirectOffsetOnAxis(
                            ap=srcT[:, off + cg:off + cg + 1], axis=0),
                    )
                scr = work.tile([P, CG, H], BF16, tag="scr")
                nc.vector.tensor_tensor(
                    out=scr[:], in0=gt[:, :, 0:H],
                    in1=prepb[b][:, None, :].to_broadcast([P, CG, H]),
                    op=AO.mult)
                ssc = work.tile([P, CG], F32, tag="ssc")
                nc.vector.tensor_reduce(out=ssc[:], in_=scr[:],
                                        axis=mybir.AxisListType.X, op=AO.add)
                ssb = work.tile([P, CG], BF16, tag="ssb")
                nc.vector.tensor_copy(out=ssb[:], in_=ssc[:])
                drp = psum.tile([P, P], BF16, tag="drp")
                nc.tensor.transpose(
                    drp[:], dsb16[:, 3 + b, t:t + 1].to_broadcast([P, P]), idb)
                dre = work.tile([P, P], BF16, tag="dre")
                nc.scalar.copy(dre[:], drp[:])
                tw = work.tile([P, CG, P], BF16, tag="tw")
                nc.vector.tensor_tensor(
                    out=tw[:],
                    in0=dre[:, None, :].to_broadcast([P, CG, P]),
                    in1=ssb[:, :, None].to_broadcast([P, CG, P]),
                    op=AO.add)
                t02 = work.tile([P, CG, P], BF16, tag="t02")
                nc.vector.tensor_scalar_mul(t02[:], tw[:], NEG_SLOPE)
                tm = work.tile([P, CG, P], BF16, tag="tm")
                nc.vector.tensor_tensor(out=tm[:], in0=tw[:], in1=t02[:],
                                        op=AO.max)
                ex = work.tile([P, CG, P], BF16, tag="ex")
                nc.scalar.activation(ex[:], tm[:], AF.Exp)
                oh = work.tile([P, CG, P], BF16, tag="oh")
                nc.vector.tensor_tensor(
                    out=oh[:],
                    in0=dstT[:, off:off + CG, None].to_broadcast([P, CG, P]),
                    in1=iotab[:, None, :].to_broadcast([P, CG, P]),
                    op=AO.is_equal)
                sm_ = work.tile([P, CG, P], BF16, tag="smat")
                nc.vector.tensor_tensor(out=sm_[:], in0=ex[:], in1=oh[:],
                                        op=AO.mult)
                ssf = work.tile([P, P], BF16, tag="ssf")
                nc.vector.tensor_scalar_mul(ssf[:], idb, wself[:, b:b + 1])
                yps = psy.tile([P, GROW], F32, tag="yps")
                for cg in range(CG):
                    nc.tensor.matmul(yps[:], lhsT=sm_[:, cg, :],
                                     rhs=gt[:, cg, :],
                                     start=(cg == 0), stop=False)
                nc.tensor.matmul(yps[:], lhsT=ssf[:], rhs=gts[:],
                                 start=False, stop=True)
                rec = work.tile([P, 1], F32, tag="rec")
                nc.vector.reciprocal(rec[:], yps[:, FLAG:FLAG + 1])
                y = work.tile([P, H], BF16, tag="y")
                nc.vector.tensor_scalar_mul(y[:], yps[:, 0:H], rec[:])
                yT = work.tile([P, 2, P], BF16, tag="yT")
                for k in range(2):
                    tps = psum.tile([P, P], BF16, tag="tps")
                    nc.tensor.transpose(tps[:], y[:, k * P:(k + 1) * P], idb)
                    nc.vector.tensor_copy(out=yT[:, k, :], in_=tps[:])
                zps = psy.tile([P, H], F32, tag="zps")
                for k in range(2):
                    nc.tensor.matmul(zps[:], lhsT=yT[:, k, :],
                                     rhs=wc[:, b, k, :],
                                     start=(k == 0), stop=(k == 1))
                return zps

            def heron(vrs, w, tagp):
                # sqrt via 3 Heron iterations, then reciprocal
                sh = epi.tile([P, w], F32, tag=tagp + "sh")
                nc.vector.tensor_scalar(sh[:], vrs[:], 0.5, 0.5,
                                        AO.mult, AO.add)
                for _ in range(3):
                    hu = epi.tile([P, w], F32, tag=tagp + "hu")
                    nc.vector.reciprocal(hu[:], sh[:])
                    ht = epi.tile([P, w], F32, tag=tagp + "ht")
                    nc.vector.tensor_tensor(out=ht[:], in0=vrs[:], in1=hu[:],
                                            op=AO.mult)
                    h2 = epi.tile([P, w], F32, tag=tagp + "h2")
                    nc.vector.tensor_tensor(out=h2[:], in0=sh[:], in1=ht[:],
                                            op=AO.add)
                    sh = epi.tile([P, w], F32, tag=tagp + "sh")
                    nc.vector.tensor_scalar_mul(sh[:], h2[:], 0.5)
                ri = epi.tile([P, w], F32, tag=tagp + "ri")
                nc.vector.reciprocal(ri[:], sh[:])
                return ri

            gi = 0
            t0 = 0
            while t0 < T:
                NG = min(GT, T - t0)
                W4 = 4 * NG
                sums = epi.tile([P, 4 * GT], F32, tag="sums")
                sqs = epi.tile([P, 4 * GT], F32, tag="sqs")
                vss = []
                wselfs = []
                for g in range(NG):
                    t = t0 + g
                    gts = work.tile([P, GROW], BF16, tag="gts")
                    nc.sync.dma_start(out=gts[:],
                                      in_=xr_in.ap()[t * P:(t + 1) * P, :])
                    ws1 = work.tile([P, 3], F32, tag="ws1")
                    nc.vector.tensor_tensor(out=ws1[:],
                                            in0=dsall[:, 0:3, t:t + 1],
                                            in1=dsall[:, 3:6, t:t + 1],
                                            op=AO.add)
                    ws2 = work.tile([P, 3], F32, tag="ws2")
                    nc.vector.tensor_scalar_mul(ws2[:], ws1[:], NEG_SLOPE)
                    ws3 = work.tile([P, 3], F32, tag="ws3")
                    nc.vector.tensor_tensor(out=ws3[:], in0=ws1[:],
                                            in1=ws2[:], op=AO.max)
                    wself = work.tile([P, 3], F32, tag="wself")
                    nc.scalar.activation(wself[:], ws3[:], AF.Exp)
                    wselfs.append(wself)

                    vs = epi.tile([P, 4, H], F32, tag=f"vs{g}")
                    vss.append(vs)
                    for b in range(3):
                        zps = branch_z(t, b, gi, gts, wself)
                        gi += 1
                        epi_a(zps, b, vs, sums[:, 4 * g:],
                              sqs[:, 4 * g:])
                    zps = psy.tile([P, H], F32, tag="zps")
                    for k in range(2):
                        nc.tensor.matmul(
                            zps[:], lhsT=xt[:, k, t * P:(t + 1) * P],
                            rhs=wc[:, 3, k, :], start=(k == 0), stop=(k == 1))
                    epi_a(zps, 3, vs, sums[:, 4 * g:], sqs[:, 4 * g:])

                # batched LN stats for NG tiles x 4 branches
                mus = epi.tile([P, 4 * GT], F32, tag="mus")
                nc.vector.tensor_scalar_mul(mus[:, :W4], sums[:, :W4],
                                            1.0 / H)
                m2s = epi.tile([P, 4 * GT], F32, tag="m2s")
                nc.vector.tensor_scalar(m2s[:, :W4], sqs[:, :W4], 1.0 / H,
                                        LN_EPS, AO.mult, AO.add)
                mqs = epi.tile([P, 4 * GT], F32, tag="mqs")
                nc.scalar.activation(mqs[:, :W4], mus[:, :W4], AF.Square)
                vrs = epi.tile([P, 4 * GT], F32, tag="vrs")
                nc.vector.tensor_tensor(out=vrs[:, :W4], in0=m2s[:, :W4],
                                        in1=mqs[:, :W4], op=AO.subtract)
                ris = heron(vrs[:, :W4], W4, "b")
                rsw = epi.tile([P, 4 * GT], F32, tag="rsw")
                nc.vector.tensor_tensor(out=rsw[:, :W4], in0=ris[:],
                                        in1=swg[:, :W4], op=AO.mult)

                # normalize + relu + combine + fusion matmul per tile
                fvs = []
                fsqs = epi.tile([P, GT], F32, tag="fsqs")
                fsumcat = epi.tile([P, GT], F32, tag="fsumcat")
                for g in range(NG):
                    t = t0 + g
                    vs = vss[g]
                    comb = work.tile([P, H], F32, tag="comb")
                    for b in range(4):
                        c0 = 4 * g + b
                        sc2 = rsw if gs_ones[b] else ris
                        t1 = epi.tile([P, H], F32, tag="t1")
                        nc.vector.tensor_scalar(t1[:], vs[:, b, :],
                                                mus[:, c0:c0 + 1],
                                                sc2[:, c0:c0 + 1],
                                                AO.subtract, AO.mult)
                        cur = t1
                        if not gs_ones[b]:
                            t2 = epi.tile([P, H], F32, tag="t2")
                            nc.vector.tensor_tensor(out=t2[:], in0=cur[:],
                                                    in1=gssw[:, b, :],
                                                    op=AO.mult)
                            cur = t2
                        if not bt_zero[b]:
                            t3 = epi.tile([P, H], F32, tag="t3")
                            nc.vector.tensor_tensor(out=t3[:], in0=cur[:],
                                                    in1=btsw[:, b, :],
                                                    op=AO.add)
                            cur = t3
                        if b == 0:
                            nc.scalar.activation(comb[:], cur[:], AF.Relu)
                        else:
                            t4 = epi.tile([P, H], F32, tag="t4")
                            nc.scalar.activation(t4[:], cur[:], AF.Relu)
                            nc.vector.tensor_tensor(out=comb[:], in0=comb[:],
                                                    in1=t4[:], op=AO.add)
                    cb16 = work.tile([P, H], BF16, tag="cb16")
                    nc.vector.tensor_copy(out=cb16[:], in_=comb[:])
                    cT = work.tile([P, 2, P], BF16, tag="cT")
                    for k in range(2):
                        tps = psum.tile([P, P], BF16, tag="tps")
                        nc.tensor.transpose(tps[:], cb16[:, k * P:(k + 1) * P],
                                            idb)
                        nc.vector.tensor_copy(out=cT[:, k, :], in_=tps[:])
                    fps = psy.tile([P, H], F32, tag="zps")
                    for k in range(2):
                        nc.tensor.matmul(fps[:], lhsT=cT[:, k, :],
                                         rhs=wc[:, 4, k, :],
                                         start=(k == 0), stop=(k == 1))
                    fv = epi.tile([P, H], F32, tag=f"fv{g}")
                    fvs.append(fv)
                    if bf_zero:
                        nc.scalar.activation(fv[:], fps[:], AF.Copy,
                                             accum_out=fsumcat[:, g:g + 1])
                    else:
                        nc.vector.tensor_tensor(out=fv[:], in0=fps[:],
                                                in1=fin[:, 2, :], op=AO.add)
                        nc.vector.tensor_reduce(out=fsumcat[:, g:g + 1],
                                                in_=fv[:],
                                                axis=mybir.AxisListType.X,
                                                op=AO.add)
                    sqd2 = epi.tile([P, H], F32, tag="sqd")
                    nc.scalar.activation(sqd2[:], fv[:], AF.Square,
                                         accum_out=fsqs[:, g:g + 1])

                # batched final-LN stats
                fmus = epi.tile([P, GT], F32, tag="fmus")
                nc.vector.tensor_scalar_mul(fmus[:, :NG], fsumcat[:, :NG],
                                            1.0 / H)
                fm2 = epi.tile([P, GT], F32, tag="fm2")
                nc.vector.tensor_scalar(fm2[:, :NG], fsqs[:, :NG], 1.0 / H,
                                        LN_EPS, AO.mult, AO.add)
                fmq = epi.tile([P, GT], F32, tag="fmq")
                nc.scalar.activation(fmq[:, :NG], fmus[:, :NG], AF.Square)
                fvr = epi.tile([P, GT], F32, tag="fvr")
                nc.vector.tensor_tensor(out=fvr[:, :NG], in0=fm2[:, :NG],
                                        in1=fmq[:, :NG], op=AO.subtract)
                fri = heron(fvr[:, :NG], NG, "f")
                for g in range(NG):
                    t = t0 + g
                    ft1 = epi.tile([P, H], F32, tag="ft1")
                    nc.vector.tensor_scalar(ft1[:], fvs[g][:],
                                            fmus[:, g:g + 1], fri[:, g:g + 1],
                                            AO.subtract, AO.mult)
                    fcur = ft1
                    if not gf_ones:
                        ft2 = epi.tile([P, H], F32, tag="ft2")
                        nc.vector.tensor_tensor(out=ft2[:], in0=fcur[:],
                                                in1=fin[:, 0, :], op=AO.mult)
                        fcur = ft2
                    if not btf_zero:
                        ft3 = epi.tile([P, H], F32, tag="ft3")
                        nc.vector.tensor_tensor(out=ft3[:], in0=fcur[:],
                                                in1=fin[:, 1, :], op=AO.add)
                        fcur = ft3
                    osb = work.tile([P, H], F32, tag="osb")
                    nc.scalar.activation(osb[:], fcur[:], AF.Relu)
                    nc.sync.dma_start(out=out.ap()[t * P:(t + 1) * P, :],
                                      in_=osb[:])
                t0 += NG

    nc.compile()
    return nc


# --------------------------------------------------------------------------
# host side
# --------------------------------------------------------------------------

def kernel(x, edge_index, edge_attr, Wg, a_src, a_dst, bg, Ws, bs, gs, betas,
           W1, b1, W2, b2, Wf, bf, gf, betaf):
    global LAST_EXEC_NS
    x = np.asarray(x, np.float32)
    N = x.shape[0]
    R = int(np.ceil(N / NCORES / P)) * P
    T = R // P
    NPAD = NCORES * R
    DUMMY_PAD = NPAD
    DUMMY_SELF = NPAD + 1
    GR = int(np.ceil((NPAD + 2) / P)) * P

    Wg = np.asarray(Wg, np.float64)
    a_src_ = np.asarray(a_src, np.float64)
    a_dst_ = np.asarray(a_dst, np.float64)
    bg = np.asarray(bg, np.float64)
    Ws_ = np.asarray(Ws, np.float64)
    bs_ = np.asarray(bs, np.float64)

    p = np.stack([Wg[i] @ a_src_[i] for i in range(3)])
    q = np.stack([Wg[i] @ a_dst_[i] for i in range(3)])
    C = np.stack([Wg[i] @ Ws_[i] for i in range(3)])
    cb = np.stack([bg[i] @ Ws_[i] + bs_[i] for i in range(3)])

    # gather table (replicated)
    gxt = np.zeros((GR, GROW), dtype=ml_dtypes.bfloat16)
    gxt[:N, :H] = _bf(x)
    gxt[:N, FLAG] = 1.0
    gxt[DUMMY_SELF, FLAG] = 1.0

    # ---- edge bucketing ----
    src = np.asarray(edge_index)[0].astype(np.int64)
    dst = np.asarray(edge_index)[1].astype(np.int64)
    attr = np.asarray(edge_attr).astype(np.int64)
    keep = attr < 3
    ks = src[keep]
    kd = dst[keep]
    ka = attr[keep]
    core_of = kd // R
    tl = (kd - core_of * R) // P
    jl = (kd - core_of * R) % P
    # group id = ((core*T + t)*3 + b)
    gid = (core_of * T + tl) * 3 + ka
    order = np.argsort(gid, kind="stable")
    gid_s, ks_s, jl_s = gid[order], ks[order], jl[order]
    counts = np.bincount(gid_s, minlength=NCORES * T * 3).reshape(NCORES, T, 3)
    bounds = np.concatenate([[0], np.cumsum(
        counts.reshape(-1))]).astype(np.int64)

    # chunk counts per (t, b): max over cores (self-loops go direct)
    cgs = np.maximum(np.ceil(counts.max(axis=0) / P), 1).astype(np.int64)
    cgs_tb = cgs.reshape(-1)  # (t, b) order
    totc = int(cgs_tb.sum())
    offs = np.concatenate([[0], np.cumsum(cgs_tb)]).astype(np.int64)

    srcT = np.full((NCORES, P, totc), DUMMY_PAD, dtype=np.int32)
    dstT = np.zeros((NCORES, P, totc), dtype=np.float32)
    for c in range(NCORES):
        for t in range(T):
            for b in range(3):
                g = (c * T + t) * 3 + b
                lo, hi = bounds[g], bounds[g + 1]
                e_src = ks_s[lo:hi]
                e_jl = jl_s[lo:hi]
                off = offs[t * 3 + b]
                CG = cgs[t, b]
                ne = len(e_src)
                buf_s = np.full(CG * P, DUMMY_PAD, dtype=np.int64)
                buf_j = np.zeros(CG * P, dtype=np.int64)
                buf_s[:ne] = e_src
                buf_j[:ne] = e_jl
                srcT[c, :, off:off + CG] = buf_s.reshape(CG, P).T
                dstT[c, :, off:off + CG] = buf_j.reshape(CG, P).T

    # ---- constant packs ----
    def rep(v):  # replicate a [H] vector across partitions
        return np.tile(np.asarray(v, np.float32)[None, :], (P, 1))

    wcs = [C[0], C[1], C[2], np.asarray(Ws_[3]), np.asarray(Wf, np.float64)]
    wc = np.zeros((P, 5, 2, H), dtype=ml_dtypes.bfloat16)
    for ci, M in enumerate(wcs):
        for k in range(2):
            wc[:, ci, k, :] = _bf(M[k * P:(k + 1) * P, :])
    prep = np.zeros((P, 3, H), dtype=ml_dtypes.bfloat16)
    for b in range(3):
        prep[:, b, :] = _bf(p[b])[None, :]
    pqa = np.zeros((P, 2, 6), dtype=ml_dtypes.bfloat16)
    for k in range(2):
        for j in range(3):
            pqa[:, k, j] = _bf(p[j][k * P:(k + 1) * P])
            pqa[:, k, 3 + j] = _bf(q[j][k * P:(k + 1) * P])
    gs4 = np.stack([rep(np.asarray(gs)[b]) for b in range(4)], axis=1)
    bt4 = np.stack([rep(np.asarray(betas)[b]) for b in range(4)], axis=1)
    cb4 = np.stack([rep(cb[0]), rep(cb[1]), rep(cb[2]),
                    rep(np.asarray(bs)[3])], axis=1)
    fin = np.stack([rep(gf), rep(betaf), rep(bf)], axis=1)
    w1a = np.zeros((P, 2, P), np.float32)
    W1a = np.asarray(W1, np.float32)
    for k in range(2):
        w1a[:, k, :] = W1a[k * P:(k + 1) * P, :]
    b1a = np.asarray(b1, np.float32).reshape(P, 1)
    w2a = np.asarray(W2, np.float32)  # [128, 4]
    b2a = np.zeros((P, 1), np.float32)
    b2a[:4, 0] = np.asarray(b2, np.float32)
    auxa = np.zeros((P, 2, P), np.float32)
    auxa[:, 0, :] = np.eye(P, dtype=np.float32)
    auxa[:, 1, :] = np.arange(P, dtype=np.float32)[None, :]
    auxb = np.zeros((P, 2, P), dtype=ml_dtypes.bfloat16)
    auxb[:, 0, :] = np.eye(P, dtype=np.float32)
    auxb[:, 1, :] = np.arange(P, dtype=np.float32)[None, :]

    xpad = np.zeros((NPAD, H), np.float32)
    xpad[:N] = x

    cb_zero = tuple(bool(np.all(cb4[:, b, :] == 0)) for b in range(4))
    gs_ones = tuple(bool(np.all(gs4[:, b, :] == 1)) for b in range(4))
    bt_zero = tuple(bool(np.all(bt4[:, b, :] == 0)) for b in range(4))
    gf_ones = bool(np.all(fin[:, 0, :] == 1))
    btf_zero = bool(np.all(fin[:, 1, :] == 0))
    bf_zero = bool(np.all(fin[:, 2, :] == 0))
    flags = (cb_zero, gs_ones, bt_zero, gf_ones, bf_zero, btf_zero)

    key = (T, GR, N, tuple(int(v) for v in cgs_tb), flags)
    if key not in _GRAPH_CACHE:
        _GRAPH_CACHE[key] = _build(T, GR, list(cgs_tb), N, flags)
    nc = _GRAPH_CACHE[key]

    in_maps = []
    for c in range(NCORES):
        xo = xpad[c * R:(c + 1) * R]  # [R, H]
        xtc = np.zeros((P, 2, R), dtype=ml_dtypes.bfloat16)
        xoT = _bf(xo).T  # [H, R]
        for k in range(2):
            xtc[:, k, :] = xoT[k * P:(k + 1) * P, :]
        xrc = gxt[c * R:(c + 1) * R].copy()
        xrc[:, FLAG] = 1.0
        in_maps.append({
            "gx": gxt,
            "xr": xrc,
            "xt": xtc.reshape(P, 2 * R),
            "wc": wc.reshape(P, 5 * 2 * H),
            "prep": prep.reshape(P, 3 * H),
            "pq": pqa.reshape(P, 2 * 6),
            "gsb": gs4.reshape(P, 4 * H),
            "btb": bt4.reshape(P, 4 * H),
            "cbb": cb4.reshape(P, 4 * H),
            "fin": fin.reshape(P, 3 * H),
            "w1": w1a.reshape(P, 2 * P),
            "b1": b1a,
            "w2": w2a,
            "b2": b2a,
            "aux": auxa.reshape(P, 2 * P),
            "auxb": auxb.reshape(P, 2 * P),
            "srcT": srcT[c],
            "dstT": dstT[c].astype(ml_dtypes.bfloat16),
        })

    trace = os.environ.get("KERNEL_TRACE", "0") == "1"
    res = run_bass_kernel_spmd(nc, in_maps, core_ids=list(range(NCORES)),
                               trace=trace)
    LAST_EXEC_NS = res.exec_time_ns
    global LAST_RES
    LAST_RES = res
    full = np.concatenate([res.results[c]["out"] for c in range(NCORES)],
                          axis=0)
    return full[:N].astype(np.float32)
